# revision 16
# baseline (speedup 1.0000x reference)
"""Behler-Parrinello NN energy kernel for 8 Trainium2 NeuronCores — v2.

Strategy (per core: 125k H + 125k O atoms, data-parallel over atoms)
--------------------------------------------------------------------
Host: molecule->partition snake plan (unchanged from v1), atoms laid out
in a [128 x 992] slot grid per element; features cast to fp8 (e4m3,
TRN FP8_EXP4) feature-major — halves HBM traffic vs fp16. A small
number of leading macro-pairs per element (K16) can stay fp16 for
accuracy margin.

Device per macro-pair (4096 atoms, [128, 4096] x-tile):
  * L1: mixed-precision matmuls — fp16 W1 (exact weights) x fp8 x,
    2-up column packing, psum [128, 1024] per j2; tanh on the scalar
    engine (bias b1) -> h1 fp16.
  * L2: block-diag W2 fp16 matmuls into [128, 512] psum chunks (1 bank);
    tanh is approximated by a custom 7-stage DVE op:
      h2 = p(z) = ((c2 z^2 + c1) z^2 + c0) * z,  z = p2 + b2
    with coefficients fit on-the-fly (host) to the actual z2 sample via
    weighted least squares — frees the scalar engine of 1/3 of its work.
  * L3: stationary-h2 matmuls: lhsT = h2 128-col slice, rhs = w3s
    [128, 4] (4-block column-packed W3) -> E[atom-partition, 4 quads]
    directly into a compact [128, 992] psum E tile (no 102us DVE
    segmented reduce, no transpose pass).
  * E psum -> SBUF fp16 copy once per bank; gpsimd local_scatter batches
    + vector adds accumulate molecule bins as in v1.
Host merges bins (float64 bincount) and adds count*b3.
"""

import sys

if "/opt/trn_rl_repo" not in sys.path:
    sys.path.insert(0, "/opt/trn_rl_repo")

import numpy as np
import ml_dtypes

# ---------------------------------------------------------------- constants
N_CORES = 8
N_MOL = 100_000
N_FEAT = 128
N_ATOMS = 1_000_000          # per element, global
APC = N_ATOMS // N_CORES     # atoms per core per element (125000)

T_COLS = 992                 # slot columns per partition per element
PAIRS = T_COLS // 32         # macro pairs per element (31)
DMA_COLS = 4096              # xt columns per macro pair
SLOTS = 128 * T_COLS         # slots per core per element (126976)
NB = 4                       # scatter batches per element
BW = T_COLS // NB            # columns per batch (248)
N_BINS = 1280                # bins per partition (H: [0,640), O: [640,1280))
BIN_HALF = 640
K16 = 0                      # leading fp16 macro-pairs per element (accuracy knob)

E4NP = ml_dtypes.float8_e4m3

_CACHE = {}


# ================================================================ DVE op
def _register_poly_tanh():
    """Custom DVE op: out = ((imm2*t + s1)*t + C3)*(in0 + s0), t=(in0+s0)^2.
    s0 = b2 bias [P,1], s1 = c1 [P,1], imm2 = c2 (baked), C3 (via in1) = c0."""
    from concourse import dve_ops as DO

    for op in DO.OPS:
        if op.name == "POLY_TANH_B":
            return op

    from concourse.dve_spec import (
        Spec, Src0, C0, C1, C2, C3, lower, _spill_c3_to_src1,
    )
    from concourse.dve_table_gen import DveOpSpec

    u = Src0 + C0
    t = u * u
    body = _spill_c3_to_src1(((C2 * t + C1) * t + C3) * u)

    def ref(in0, in1, s0, s1, imm2):
        uu = in0.astype(np.float32) + s0
        tt = uu * uu
        c0 = in1.reshape(in1.shape[0], -1)[:, :1]
        return ((imm2 * tt + s1) * tt + c0) * uu

    spec = Spec(body=body, reference=ref)
    shas = {}
    for ver in ("v3", "v4"):
        s = DveOpSpec(name="POLY_TANH_B", opcode=0, uops=lower(spec, ver=ver),
                      rd1_en=True)
        shas[ver] = s.sha(ver)
    op = DO.DveOp("POLY_TANH_B", spec, subdim=False, uops_sha=shas)
    DO.OPS.append(op)
    DO.CUSTOM_DVE_SPECS[op.name] = op.spec
    DO._SUB_OPCODE_FOR_NAME[op.name] = DO._CUSTOM_DVE_ROW_BASE + len(DO.OPS) - 1
    return op


# ================================================================ device IR
def _build_nc(c2_by_elem, k16):
    import concourse.bacc as bacc
    import concourse.mybir as mybir
    from concourse.tile import TileContext

    dt = mybir.dt
    f8, f16, f32, i16 = dt.float8e4, dt.float16, dt.float32, dt.int16
    Tanh = mybir.ActivationFunctionType.Tanh
    ADD = mybir.AluOpType.add

    poly_op = _register_poly_tanh()

    nc = bacc.Bacc("TRN2", target_bir_lowering=False, debug=False)

    n8 = (PAIRS - k16) * DMA_COLS
    n16 = k16 * DMA_COLS
    xt8 = {e: nc.dram_tensor(f"xt8_{e}", [128, n8], f8, kind="ExternalInput")
           for e in ("h", "o")}
    xt16 = {}
    if k16:
        xt16 = {e: nc.dram_tensor(f"xt16_{e}", [128, n16], f16,
                                  kind="ExternalInput")
                for e in ("h", "o")}
    wpk = {e: nc.dram_tensor(f"wpk_{e}", [128, 132], f16, kind="ExternalInput")
           for e in ("h", "o")}
    bpk = {e: nc.dram_tensor(f"bpk_{e}", [128, 4], f32, kind="ExternalInput")
           for e in ("h", "o")}
    q_idx = nc.dram_tensor("q_idx", [128, 2 * T_COLS], i16, kind="ExternalInput")
    out_acc = nc.dram_tensor("out_acc", [128, N_BINS], f32, kind="ExternalOutput")

    with TileContext(nc) as tc:
        with (
            tc.tile_pool(name="wpool", bufs=1) as wpool,
            tc.tile_pool(name="xpool8", bufs=5) as xpool8,
            tc.tile_pool(name="xpool16", bufs=2) as xpool16,
            tc.tile_pool(name="hpool", bufs=4) as hpool,
            tc.tile_pool(name="h2pool", bufs=3) as h2pool,
            tc.tile_pool(name="epool", bufs=1) as epool,
            tc.tile_pool(name="spool", bufs=3) as spool,
            tc.tile_pool(name="ps1", bufs=2, space="PSUM") as ps1,
            tc.tile_pool(name="ps2", bufs=2, space="PSUM") as ps2,
            tc.tile_pool(name="psE", bufs=1, space="PSUM") as psE,
        ):
            # --- persistent tiles
            E = epool.tile([128, 2 * T_COLS], f16, tag="E")
            Q = epool.tile([128, 2 * T_COLS], i16, tag="Q")
            acc = epool.tile([128, N_BINS], f32, tag="acc")

            # warm the act table without waiting on the big acc memset
            warm = epool.tile([128, 2], f32, tag="warm")
            nc.vector.memset(warm[:], 0.0)
            nc.scalar.activation(warm[:, 0:1], warm[:, 1:2], Tanh)

            nc.vector.memset(acc[:], 0.0)
            nc.gpsimd.dma_start(Q[:], q_idx[:])

            wt = {}
            for e in ("h", "o"):
                wtile = wpool.tile([128, 132], f16, tag=f"wp{e}", name=f"wp{e}")
                btile = wpool.tile([128, 4], f32, tag=f"bp{e}", name=f"bp{e}")
                nc.sync.dma_start(wtile[:], wpk[e][:])
                nc.sync.dma_start(btile[:], bpk[e][:])
                wt[e] = {
                    "w1": wtile[:, 0:64],
                    "w2s": wtile[:, 64:128],
                    "w3s": wtile[:, 128:132],
                    "b1s": btile[:, 0:1],
                    "b2s": btile[:, 1:2],
                    "c0s": btile[:, 2:3],
                    "c1s": btile[:, 3:4],
                }

            def emit_chunks(nc, ei, jp_done, Eps):
                """Emit E-copies + scatter batches whose columns are complete
                once pair jp_done's L3 matmuls have been emitted."""
                for c_lo, c_hi, batches in (
                    (0, 512, ((0, 248, "dve"), (248, 248, "dve"))),
                    (512, 768, ((496, 248, "dve"),)),
                    (768, 960, ((744, 216, "dve"),)),
                    (960, T_COLS, ((960, 32, "dve"),)),
                ):
                    if jp_done != (c_hi + 31) // 32 - 1:
                        continue
                    with nc.allow_low_precision("fp16 energies"):
                        nc.vector.tensor_copy(
                            E[:, ei * T_COLS + c_lo:ei * T_COLS + c_hi],
                            Eps[:, c_lo:c_hi],
                        )
                    for r0, w, eng in batches:
                        _scatter_batch(nc, spool, acc, E, Q, ei, r0, w,
                                       f16, ADD, eng)
                if jp_done == PAIRS - 1:
                    nc.gpsimd.dma_start(
                        out_acc[:, ei * BIN_HALF:(ei + 1) * BIN_HALF],
                        acc[:, ei * BIN_HALF:(ei + 1) * BIN_HALF],
                    )

            for ei, e in enumerate(("h", "o")):
                W = wt[e]
                Eps = psE.tile([128, T_COLS], f32, tag="Eps", name=f"Eps{e}")
                pending_l3 = []
                for jp in range(PAIRS):
                    if jp < k16:
                        xtile = xpool16.tile([128, DMA_COLS], f16, tag="xt16",
                                             name=f"x{e}{jp}")
                        src = xt16[e][:, jp * DMA_COLS:(jp + 1) * DMA_COLS]
                    else:
                        xtile = xpool8.tile([128, DMA_COLS], f8, tag="xt8",
                                            name=f"x{e}{jp}")
                        o8 = (jp - k16) * DMA_COLS
                        src = xt8[e][:, o8:o8 + DMA_COLS]
                    if ei == 0 and jp == 0:
                        # split the very first DMA so L1 can start sooner
                        for q4 in range(4):
                            nc.sync.dma_start(
                                xtile[:, 1024 * q4:1024 * (q4 + 1)],
                                src[:, 1024 * q4:1024 * (q4 + 1)])
                    else:
                        nc.sync.dma_start(xtile[:], src)

                    # ---- L1 + tanh (scalar engine); the previous pair's L3
                    # matmuls are interleaved between L1 matmuls so their
                    # LDWEIGHTS hide under L1's 512-col streams.
                    h1s = []
                    for j2 in range(2):
                        p1 = ps1.tile([128, 1024], f32, tag="p1",
                                      name=f"p1_{e}{jp}_{j2}")
                        for sb in range(2):
                            for blk in range(2):
                                o = 2048 * j2 + 1024 * sb + 512 * blk
                                nc.tensor.matmul(
                                    p1[64 * blk:64 * blk + 64,
                                       512 * sb:512 * (sb + 1)],
                                    W["w1"],
                                    xtile[:, o:o + 512],
                                    tile_position=(0, 64 * blk),
                                )
                                if pending_l3:
                                    pending_l3.pop(0)()
                        h1 = hpool.tile([128, 1024], f16, tag="h1",
                                        name=f"h1_{e}{jp}_{j2}")
                        nc.scalar.activation(h1[:], p1[:], Tanh, bias=W["b1s"])
                        h1s.append(h1)
                    if jp > 0:
                        emit_chunks(nc, ei, jp - 1, Eps)

                    # ---- L2 + poly tanh (DVE); remaining L3 pops fill the
                    # PE window while Act finishes h1(j2=1)
                    h2 = h2pool.tile([128, 1024], f16, tag="h2",
                                     name=f"h2_{e}{jp}")
                    for j2 in range(2):
                        p2 = ps2.tile([128, 512], f32, tag="p2",
                                      name=f"p2_{e}{jp}_{j2}")
                        for sb in range(2):
                            nc.tensor.matmul(
                                p2[64 * sb:64 * sb + 64, 0:512],
                                W["w2s"],
                                h1s[j2][:, 512 * sb:512 * (sb + 1)],
                                tile_position=(0, 64 * sb),
                            )
                        with nc.allow_low_precision("poly tanh fp16 h2"):
                            nc.vector._custom_dve(
                                poly_op,
                                out=h2[:, 512 * j2:512 * (j2 + 1)],
                                in0=p2[:],
                                in1=W["c0s"],
                                s0=W["b2s"],
                                s1=W["c1s"],
                                imm2=float(c2_by_elem[e]),
                            )

                    # ---- L3 (deferred one pair): compact E
                    def make_l3(jp, j2, s, h2):
                        col = 32 * jp + 16 * j2 + 4 * s

                        def emit():
                            nc.tensor.matmul(
                                Eps[:, col:col + 4],
                                h2[:, 512 * j2 + 128 * s:
                                   512 * j2 + 128 * s + 128],
                                W["w3s"],
                                start=(col in (0, 512)),
                                stop=(col in (508, 988)),
                                skip_group_check=True,
                            )
                        return emit

                    pending_l3 = [make_l3(jp, j2, s, h2)
                                  for j2 in range(2) for s in range(4)]

                # element drain
                for fn in pending_l3:
                    fn()
                pending_l3 = []
                emit_chunks(nc, ei, PAIRS - 1, Eps)

    nc.compile()
    return nc


def _scatter_batch(nc, spool, acc, E, Q, ei, r0, w, f16, ADD, eng):
    """Scatter E cols [r0, r0+w) of element ei into bins, add into acc."""
    import concourse.mybir as mybir

    c0 = ei * T_COLS + r0
    S = spool.tile([128, BIN_HALF], f16, tag="S", name=f"S{ei}_{r0}")
    nc.gpsimd.local_scatter(
        S[:], E[:, c0:c0 + w], Q[:, c0:c0 + w],
        channels=128, num_elems=BIN_HALF, num_idxs=w,
    )
    a = acc[:, ei * BIN_HALF:(ei + 1) * BIN_HALF]
    if eng == "dve":
        nc.vector.tensor_tensor(a, a, S[:], op=ADD)
    else:
        nc.gpsimd.tensor_tensor(a, a, S[:], op=ADD)


# ================================================================ host plan
def _plan_element(m):
    """Plan one (core, element): molecule->partition, atom->slot, bins.

    m: int32 [n] molecule index per atom (core's shard).
    Returns (perm, q, bin_mol, bin_p, bin_id): perm int64 [SLOTS] source atom
    per (col*128+p) slot (pads -> 0), q int16 [128, T_COLS] bin per slot
    (-1 for pads), bin_* arrays for the host-side merge.
    """
    n = m.shape[0]
    cnt = np.bincount(m, minlength=N_MOL)
    present = np.flatnonzero(cnt)
    order = present[np.argsort(-cnt[present], kind="stable")]
    r = np.arange(order.size)
    pat = r % 256
    p_of_rank = np.where(pat < 128, pat, 255 - pat)
    p_assign = np.full(N_MOL, -1, np.int32)
    p_assign[order] = p_of_rank
    prim = np.full(N_MOL, -1, np.int32)
    o2 = np.argsort(p_of_rank, kind="stable")
    pp = p_of_rank[o2]
    starts = np.searchsorted(pp, np.arange(128))
    within = np.arange(order.size) - starts[pp]
    prim[order[o2]] = within
    n_prim = np.bincount(pp, minlength=128)

    a_sort = np.argsort(m, kind="stable")
    ms = m[a_sort]
    gstart = np.r_[0, np.flatnonzero(np.diff(ms)) + 1]
    glen = np.diff(np.r_[gstart, n])
    k = np.arange(n) - np.repeat(gstart, glen)
    level = k // NB

    bins_sorted = prim[ms].copy()
    sp_first = (level >= 1) & (k % NB == 0)
    if sp_first.any():
        sp_pos = np.flatnonzero(sp_first)
        sp_p = p_assign[ms[sp_pos]]
        so = np.argsort(sp_p, kind="stable")
        sp_sorted_p = sp_p[so]
        sp_starts = np.searchsorted(sp_sorted_p, np.arange(128))
        sp_within = np.arange(sp_pos.size) - sp_starts[sp_sorted_p]
        sp_bin = np.empty(sp_pos.size, np.int32)
        sp_bin[so] = n_prim[sp_sorted_p] + sp_within
        gid = np.cumsum(sp_first) - 1
        lvl_mask = level >= 1
        bins_sorted[lvl_mask] = sp_bin[gid[lvl_mask]]
        sp_mol = ms[sp_pos]
        sp_part = p_assign[sp_mol]
    else:
        sp_bin = np.empty(0, np.int32)
        sp_mol = np.empty(0, np.int32)
        sp_part = np.empty(0, np.int32)

    p_atom = p_assign[ms]
    o3 = np.lexsort((k, bins_sorted, p_atom))
    p3 = p_atom[o3]
    pstarts = np.searchsorted(p3, np.arange(128))
    pos = np.arange(n) - pstarts[p3]
    load = np.bincount(p3, minlength=128)
    if load.max() > T_COLS:
        raise RuntimeError(f"partition overload {load.max()} > {T_COLS}")
    nb_used = int(n_prim.max() + (np.bincount(sp_part, minlength=128).max()
                                  if sp_part.size else 0))
    if nb_used > BIN_HALF:
        raise RuntimeError(f"bins overload {nb_used} > {BIN_HALF}")

    batch = pos % NB
    col = batch * BW + pos // NB
    atom_ids = a_sort[o3]

    perm = np.zeros(SLOTS, np.int64)
    q = np.full((128, T_COLS), -1, np.int16)
    slot = col * 128 + p3
    perm[slot] = atom_ids
    q[p3, col] = bins_sorted[o3]

    bin_mol = np.concatenate([order, sp_mol])
    bin_p = np.concatenate([p_of_rank, sp_part])
    bin_id = np.concatenate([prim[order], sp_bin])
    return perm, q, bin_mol, bin_p, bin_id


# xt column for slot (p, col): the L3 emission order defines it.
def _slot_to_xt_col():
    p = np.arange(128)[:, None]
    c = np.arange(T_COLS)[None, :]
    jp = c // 32
    rr = c % 32
    j2 = rr // 16
    s = (rr % 16) // 4
    qq = rr % 4
    sb = qq // 2
    blk = qq % 2
    return (4096 * jp + 2048 * j2 + 1024 * sb + 512 * blk + 128 * s + p)


def _prep_weights(W1, b1, W2, b2, W3):
    w1 = np.ascontiguousarray(W1, np.float16)                       # [128, 64]
    w2s = np.zeros((128, 64), np.float32)                           # block-diag
    w2s[0:64, 0:32] = W2
    w2s[64:128, 32:64] = W2
    w3s = np.zeros((128, 4), np.float32)
    w3 = np.asarray(W3)[:, 0]
    for q in range(4):
        w3s[32 * q:32 * q + 32, q] = w3
    wpk = np.ascontiguousarray(
        np.hstack([w1.astype(np.float32), w2s, w3s]), np.float16)   # [128, 132]
    b1c = np.asarray(b1, np.float32).reshape(-1, 1)
    b2c = np.asarray(b2, np.float32).reshape(-1, 1)
    b1s = np.vstack([b1c, b1c]).astype(np.float32)                  # [128, 1]
    b2s = np.vstack([b2c] * 4).astype(np.float32)                   # [128, 1]
    return wpk, b1s, b2s


def _fit_poly(feats, W1, b1, W2, b2):
    """Weighted LSQ fit of deg-5 odd poly to tanh on the actual z2 sample."""
    x8 = feats[::29].astype(E4NP).astype(np.float32)
    z1 = x8 @ np.asarray(W1).astype(np.float16).astype(np.float32) + np.asarray(b1)
    h1 = np.tanh(z1).astype(np.float16).astype(np.float32)
    z2 = h1 @ np.asarray(W2).astype(np.float16).astype(np.float32) + np.asarray(b2)
    zs = z2.ravel()
    A = np.stack([zs, zs**3, zs**5], 1)
    c, *_ = np.linalg.lstsq(A, np.tanh(zs), rcond=None)
    return c.astype(np.float64)


# ================================================================ entry
def _prepare(
    feats_H, feats_O, mol_idx_H, mol_idx_O,
    W1_H, b1_H, W2_H, b2_H, W3_H,
    W1_O, b1_O, W2_O, b2_O, W3_O,
):
    feats = {"h": np.asarray(feats_H), "o": np.asarray(feats_O)}
    mols = {
        "h": np.asarray(mol_idx_H, np.int32),
        "o": np.asarray(mol_idx_O, np.int32),
    }
    raw = {
        "h": (W1_H, b1_H, W2_H, b2_H, W3_H),
        "o": (W1_O, b1_O, W2_O, b2_O, W3_O),
    }
    wts = {}
    c2 = {}
    for e in ("h", "o"):
        W1, b1, W2, b2, W3 = raw[e]
        wpk, b1s, b2s = _prep_weights(W1, b1, W2, b2, W3)
        c = _fit_poly(feats[e], W1, b1, W2, b2)
        c0s = np.full((128, 1), c[0], np.float32)
        c1s = np.full((128, 1), c[1], np.float32)
        bpk = np.ascontiguousarray(np.hstack([b1s, b2s, c0s, c1s]), np.float32)
        wts[e] = (wpk, bpk)
        c2[e] = float(c[2])

    xcol = _slot_to_xt_col()                       # [128, T_COLS]
    n16 = K16 * DMA_COLS

    in_maps = []
    merge = []
    for cidx in range(N_CORES):
        im = {}
        mg = []
        q_full = np.empty((128, 2 * T_COLS), np.int16)
        for ei, e in enumerate(("h", "o")):
            sl = slice(cidx * APC, (cidx + 1) * APC)
            perm, q, bm, bp, bid = _plan_element(mols[e][sl])
            # remap: perm indexed by (col*128 + p) -> xt-column order
            perm_xt = np.empty(SLOTS, np.int64)
            slot_idx = (np.arange(T_COLS)[None, :] * 128
                        + np.arange(128)[:, None])        # [128, T_COLS]
            perm_xt[xcol.ravel()] = perm[slot_idx.ravel()]
            g = feats[e][sl][perm_xt]                     # [SLOTS, 128] f32
            if n16:
                im[f"xt16_{e}"] = np.ascontiguousarray(
                    g[:n16].astype(np.float16).T)
            im[f"xt8_{e}"] = np.ascontiguousarray(g[n16:].astype(E4NP).T)
            del g
            off = ei * BIN_HALF
            q_full[:, ei * T_COLS:(ei + 1) * T_COLS] = q   # bins in [0, 640)
            mg.append((bm, bp, bid + off))
            im[f"wpk_{e}"], im[f"bpk_{e}"] = wts[e]
        im["q_idx"] = q_full
        in_maps.append(im)
        merge.append(mg)
    return in_maps, merge, c2


def kernel(
    feats_H, feats_O, mol_idx_H, mol_idx_O, n_molecules,
    W1_H, b1_H, W2_H, b2_H, W3_H, b3_H,
    W1_O, b1_O, W2_O, b2_O, W3_O, b3_O,
):
    from concourse import bass_utils

    in_maps, merge, c2 = _prepare(
        feats_H, feats_O, mol_idx_H, mol_idx_O,
        W1_H, b1_H, W2_H, b2_H, W3_H,
        W1_O, b1_O, W2_O, b2_O, W3_O,
    )
    key = ("nc", round(c2["h"], 9), round(c2["o"], 9), K16)
    if _CACHE.get("key") != key:
        _CACHE["nc"] = _build_nc(c2, K16)
        _CACHE["key"] = key
    nc = _CACHE["nc"]

    _CACHE["in_maps"] = in_maps
    res = bass_utils.run_bass_kernel_spmd(
        nc, in_maps, core_ids=list(range(N_CORES))
    )

    mols = {
        "h": np.asarray(mol_idx_H, np.int32),
        "o": np.asarray(mol_idx_O, np.int32),
    }
    out = np.zeros(N_MOL, np.float64)
    for c in range(N_CORES):
        acc = res.results[c]["out_acc"]
        for bm, bp, bid in merge[c]:
            out += np.bincount(
                bm, weights=acc[bp, bid].astype(np.float64), minlength=N_MOL
            )
    cnt_h = np.bincount(mols["h"], minlength=N_MOL)
    cnt_o = np.bincount(mols["o"], minlength=N_MOL)
    out += cnt_h * float(np.asarray(b3_H).reshape(-1)[0])
    out += cnt_o * float(np.asarray(b3_O).reshape(-1)[0])
    return out.astype(np.float32)


# revision 17
# speedup vs baseline: 1.1803x; 1.1803x over previous
"""Behler-Parrinello NN energy kernel for 8 Trainium2 NeuronCores — v2.

Strategy (per core: 125k H + 125k O atoms, data-parallel over atoms)
--------------------------------------------------------------------
Host: molecule->partition snake plan (unchanged from v1), atoms laid out
in a [128 x 992] slot grid per element; features cast to fp8 (e4m3,
TRN FP8_EXP4) feature-major — halves HBM traffic vs fp16. A small
number of leading macro-pairs per element (K16) can stay fp16 for
accuracy margin.

Device per macro-pair (4096 atoms, [128, 4096] x-tile):
  * L1: mixed-precision matmuls — fp16 W1 (exact weights) x fp8 x,
    2-up column packing, psum [128, 1024] per j2; tanh on the scalar
    engine (bias b1) -> h1 fp16.
  * L2: block-diag W2 fp16 matmuls into [128, 512] psum chunks (1 bank);
    tanh is approximated by a custom 7-stage DVE op:
      h2 = p(z) = ((c2 z^2 + c1) z^2 + c0) * z,  z = p2 + b2
    with coefficients fit on-the-fly (host) to the actual z2 sample via
    weighted least squares — frees the scalar engine of 1/3 of its work.
  * L3: stationary-h2 matmuls: lhsT = h2 128-col slice, rhs = w3s
    [128, 4] (4-block column-packed W3) -> E[atom-partition, 4 quads]
    directly into a compact [128, 992] psum E tile (no 102us DVE
    segmented reduce, no transpose pass).
  * E psum -> SBUF fp16 copy once per bank; gpsimd local_scatter batches
    + vector adds accumulate molecule bins as in v1.
Host merges bins (float64 bincount) and adds count*b3.
"""

import sys

if "/opt/trn_rl_repo" not in sys.path:
    sys.path.insert(0, "/opt/trn_rl_repo")

import numpy as np
import ml_dtypes

# ---------------------------------------------------------------- constants
N_CORES = 8
N_MOL = 100_000
N_FEAT = 128
N_ATOMS = 1_000_000          # per element, global
APC = N_ATOMS // N_CORES     # atoms per core per element (125000)

T_COLS = 992                 # slot columns per partition per element
PAIRS = T_COLS // 32         # macro pairs per element (31)
DMA_COLS = 4096              # xt columns per macro pair
SLOTS = 128 * T_COLS         # slots per core per element (126976)
NB = 4                       # scatter batches per element
BW = T_COLS // NB            # columns per batch (248)
N_BINS = 1280                # bins per partition (H: [0,640), O: [640,1280))
BIN_HALF = 640
K16 = 0                      # leading fp16 macro-pairs per element (accuracy knob)

E4NP = ml_dtypes.float8_e4m3

_CACHE = {}


# ================================================================ DVE op
def _register_poly_tanh():
    """Custom DVE op: out = ((imm2*t + s1)*t + C3)*(in0 + s0), t=(in0+s0)^2.
    s0 = b2 bias [P,1], s1 = c1 [P,1], imm2 = c2 (baked), C3 (via in1) = c0."""
    from concourse import dve_ops as DO

    for op in DO.OPS:
        if op.name == "POLY_TANH_B":
            return op

    from concourse.dve_spec import (
        Spec, Src0, C0, C1, C2, C3, lower, _spill_c3_to_src1,
    )
    from concourse.dve_table_gen import DveOpSpec

    u = Src0 + C0
    t = u * u
    body = _spill_c3_to_src1(((C2 * t + C1) * t + C3) * u)

    def ref(in0, in1, s0, s1, imm2):
        uu = in0.astype(np.float32) + s0
        tt = uu * uu
        c0 = in1.reshape(in1.shape[0], -1)[:, :1]
        return ((imm2 * tt + s1) * tt + c0) * uu

    spec = Spec(body=body, reference=ref)
    shas = {}
    for ver in ("v3", "v4"):
        s = DveOpSpec(name="POLY_TANH_B", opcode=0, uops=lower(spec, ver=ver),
                      rd1_en=True)
        shas[ver] = s.sha(ver)
    op = DO.DveOp("POLY_TANH_B", spec, subdim=False, uops_sha=shas)
    DO.OPS.append(op)
    DO.CUSTOM_DVE_SPECS[op.name] = op.spec
    DO._SUB_OPCODE_FOR_NAME[op.name] = DO._CUSTOM_DVE_ROW_BASE + len(DO.OPS) - 1
    return op


# ================================================================ device IR
def _build_nc(c2_by_elem, k16):
    import concourse.bacc as bacc
    import concourse.mybir as mybir
    from concourse.tile import TileContext

    dt = mybir.dt
    f8, f16, f32, i16 = dt.float8e4, dt.float16, dt.float32, dt.int16
    Tanh = mybir.ActivationFunctionType.Tanh
    ADD = mybir.AluOpType.add

    poly_op = _register_poly_tanh()

    nc = bacc.Bacc("TRN2", target_bir_lowering=False, debug=False)

    n8 = (PAIRS - k16) * DMA_COLS
    n16 = k16 * DMA_COLS
    xt8 = {e: nc.dram_tensor(f"xt8_{e}", [128, n8], f8, kind="ExternalInput")
           for e in ("h", "o")}
    xt16 = {}
    if k16:
        xt16 = {e: nc.dram_tensor(f"xt16_{e}", [128, n16], f16,
                                  kind="ExternalInput")
                for e in ("h", "o")}
    wpk = {e: nc.dram_tensor(f"wpk_{e}", [128, 132], f16, kind="ExternalInput")
           for e in ("h", "o")}
    bpk = {e: nc.dram_tensor(f"bpk_{e}", [128, 4], f32, kind="ExternalInput")
           for e in ("h", "o")}
    q_idx = nc.dram_tensor("q_idx", [128, 2 * T_COLS], i16, kind="ExternalInput")
    out_acc = nc.dram_tensor("out_acc", [128, N_BINS], f32, kind="ExternalOutput")

    with TileContext(nc) as tc:
        with (
            tc.tile_pool(name="wpool", bufs=1) as wpool,
            tc.tile_pool(name="xpool8", bufs=5) as xpool8,
            tc.tile_pool(name="xpool16", bufs=2) as xpool16,
            tc.tile_pool(name="hpool", bufs=4) as hpool,
            tc.tile_pool(name="h2pool", bufs=3) as h2pool,
            tc.tile_pool(name="epool", bufs=1) as epool,
            tc.tile_pool(name="spool", bufs=3) as spool,
            tc.tile_pool(name="ps1", bufs=2, space="PSUM") as ps1,
            tc.tile_pool(name="ps2", bufs=2, space="PSUM") as ps2,
            tc.tile_pool(name="psE", bufs=1, space="PSUM") as psE,
        ):
            # --- persistent tiles
            E = epool.tile([128, 2 * T_COLS], f16, tag="E")
            Q = epool.tile([128, 2 * T_COLS], i16, tag="Q")
            acc = epool.tile([128, N_BINS], f32, tag="acc")
            nc.vector.memset(acc[:], 0.0)

            warm = epool.tile([128, 1], f32, tag="warm")
            nc.scalar.activation(warm[:], acc[:, 0:1], Tanh)

            nc.gpsimd.dma_start(Q[:], q_idx[:])

            wt = {}
            for e in ("h", "o"):
                wtile = wpool.tile([128, 132], f16, tag=f"wp{e}", name=f"wp{e}")
                btile = wpool.tile([128, 4], f32, tag=f"bp{e}", name=f"bp{e}")
                nc.sync.dma_start(wtile[:], wpk[e][:])
                nc.sync.dma_start(btile[:], bpk[e][:])
                wt[e] = {
                    "w1": wtile[:, 0:64],
                    "w2s": wtile[:, 64:128],
                    "w3s": wtile[:, 128:132],
                    "b1s": btile[:, 0:1],
                    "b2s": btile[:, 1:2],
                    "c0s": btile[:, 2:3],
                    "c1s": btile[:, 3:4],
                }

            def emit_chunks(nc, ei, jp_done, Eps):
                """Emit E-copies + scatter batches whose columns are complete
                once pair jp_done's L3 matmuls have been emitted."""
                for c_lo, c_hi, batches in (
                    (0, 512, ((0, 248, "dve"), (248, 248, "dve"))),
                    (512, 768, ((496, 248, "dve"),)),
                    (768, 960, ((744, 216, "dve"),)),
                    (960, T_COLS, ((960, 32, "dve"),)),
                ):
                    if jp_done != (c_hi + 31) // 32 - 1:
                        continue
                    with nc.allow_low_precision("fp16 energies"):
                        nc.vector.tensor_copy(
                            E[:, ei * T_COLS + c_lo:ei * T_COLS + c_hi],
                            Eps[:, c_lo:c_hi],
                        )
                    for r0, w, eng in batches:
                        _scatter_batch(nc, spool, acc, E, Q, ei, r0, w,
                                       f16, ADD, eng)
                if jp_done == PAIRS - 1:
                    nc.gpsimd.dma_start(
                        out_acc[:, ei * BIN_HALF:(ei + 1) * BIN_HALF],
                        acc[:, ei * BIN_HALF:(ei + 1) * BIN_HALF],
                    )

            for ei, e in enumerate(("h", "o")):
                W = wt[e]
                Eps = psE.tile([128, T_COLS], f32, tag="Eps", name=f"Eps{e}")
                pending_l3 = []
                for jp in range(PAIRS):
                    if jp < k16:
                        xtile = xpool16.tile([128, DMA_COLS], f16, tag="xt16",
                                             name=f"x{e}{jp}")
                        src = xt16[e][:, jp * DMA_COLS:(jp + 1) * DMA_COLS]
                    else:
                        xtile = xpool8.tile([128, DMA_COLS], f8, tag="xt8",
                                            name=f"x{e}{jp}")
                        o8 = (jp - k16) * DMA_COLS
                        src = xt8[e][:, o8:o8 + DMA_COLS]
                    if ei == 0 and jp == 0:
                        # split the very first DMA so L1 can start sooner
                        for q4 in range(4):
                            nc.sync.dma_start(
                                xtile[:, 1024 * q4:1024 * (q4 + 1)],
                                src[:, 1024 * q4:1024 * (q4 + 1)])
                    else:
                        nc.sync.dma_start(xtile[:], src)

                    # ---- L1 + tanh (scalar engine); the previous pair's L3
                    # matmuls are interleaved between L1 matmuls so their
                    # LDWEIGHTS hide under L1's 512-col streams.
                    h1s = []
                    for j2 in range(2):
                        p1 = ps1.tile([128, 1024], f32, tag="p1",
                                      name=f"p1_{e}{jp}_{j2}")
                        for sb in range(2):
                            for blk in range(2):
                                o = 2048 * j2 + 1024 * sb + 512 * blk
                                nc.tensor.matmul(
                                    p1[64 * blk:64 * blk + 64,
                                       512 * sb:512 * (sb + 1)],
                                    W["w1"],
                                    xtile[:, o:o + 512],
                                    tile_position=(0, 64 * blk),
                                )
                                if pending_l3:
                                    pending_l3.pop(0)()
                        h1 = hpool.tile([128, 1024], f16, tag="h1",
                                        name=f"h1_{e}{jp}_{j2}")
                        nc.scalar.activation(h1[:], p1[:], Tanh, bias=W["b1s"])
                        h1s.append(h1)
                    if jp > 0:
                        emit_chunks(nc, ei, jp - 1, Eps)

                    # ---- L2 + poly tanh (DVE); remaining L3 pops fill the
                    # PE window while Act finishes h1(j2=1)
                    h2 = h2pool.tile([128, 1024], f16, tag="h2",
                                     name=f"h2_{e}{jp}")
                    for j2 in range(2):
                        p2 = ps2.tile([128, 512], f32, tag="p2",
                                      name=f"p2_{e}{jp}_{j2}")
                        for sb in range(2):
                            nc.tensor.matmul(
                                p2[64 * sb:64 * sb + 64, 0:512],
                                W["w2s"],
                                h1s[j2][:, 512 * sb:512 * (sb + 1)],
                                tile_position=(0, 64 * sb),
                            )
                        with nc.allow_low_precision("poly tanh fp16 h2"):
                            nc.vector._custom_dve(
                                poly_op,
                                out=h2[:, 512 * j2:512 * (j2 + 1)],
                                in0=p2[:],
                                in1=W["c0s"],
                                s0=W["b2s"],
                                s1=W["c1s"],
                                imm2=float(c2_by_elem[e]),
                            )

                    # ---- L3 (deferred one pair): compact E
                    def make_l3(jp, j2, s, h2):
                        col = 32 * jp + 16 * j2 + 4 * s

                        def emit():
                            nc.tensor.matmul(
                                Eps[:, col:col + 4],
                                h2[:, 512 * j2 + 128 * s:
                                   512 * j2 + 128 * s + 128],
                                W["w3s"],
                                start=(col in (0, 512)),
                                stop=(col in (508, 988)),
                                skip_group_check=True,
                            )
                        return emit

                    pending_l3 = [make_l3(jp, j2, s, h2)
                                  for j2 in range(2) for s in range(4)]

                # element drain
                for fn in pending_l3:
                    fn()
                pending_l3 = []
                emit_chunks(nc, ei, PAIRS - 1, Eps)

    nc.compile()
    return nc


def _scatter_batch(nc, spool, acc, E, Q, ei, r0, w, f16, ADD, eng):
    """Scatter E cols [r0, r0+w) of element ei into bins, add into acc."""
    import concourse.mybir as mybir

    c0 = ei * T_COLS + r0
    S = spool.tile([128, BIN_HALF], f16, tag="S", name=f"S{ei}_{r0}")
    nc.gpsimd.local_scatter(
        S[:], E[:, c0:c0 + w], Q[:, c0:c0 + w],
        channels=128, num_elems=BIN_HALF, num_idxs=w,
    )
    a = acc[:, ei * BIN_HALF:(ei + 1) * BIN_HALF]
    if eng == "dve":
        nc.vector.tensor_tensor(a, a, S[:], op=ADD)
    else:
        nc.gpsimd.tensor_tensor(a, a, S[:], op=ADD)


# ================================================================ host plan
def _plan_element(m):
    """Plan one (core, element): molecule->partition, atom->slot, bins.

    m: int32 [n] molecule index per atom (core's shard).
    Returns (perm, q, bin_mol, bin_p, bin_id): perm int64 [SLOTS] source atom
    per (col*128+p) slot (pads -> 0), q int16 [128, T_COLS] bin per slot
    (-1 for pads), bin_* arrays for the host-side merge.
    """
    n = m.shape[0]
    cnt = np.bincount(m, minlength=N_MOL)
    present = np.flatnonzero(cnt)
    order = present[np.argsort(-cnt[present], kind="stable")]
    r = np.arange(order.size)
    pat = r % 256
    p_of_rank = np.where(pat < 128, pat, 255 - pat)
    p_assign = np.full(N_MOL, -1, np.int32)
    p_assign[order] = p_of_rank
    prim = np.full(N_MOL, -1, np.int32)
    o2 = np.argsort(p_of_rank, kind="stable")
    pp = p_of_rank[o2]
    starts = np.searchsorted(pp, np.arange(128))
    within = np.arange(order.size) - starts[pp]
    prim[order[o2]] = within
    n_prim = np.bincount(pp, minlength=128)

    a_sort = np.argsort(m, kind="stable")
    ms = m[a_sort]
    gstart = np.r_[0, np.flatnonzero(np.diff(ms)) + 1]
    glen = np.diff(np.r_[gstart, n])
    k = np.arange(n) - np.repeat(gstart, glen)
    level = k // NB

    bins_sorted = prim[ms].copy()
    sp_first = (level >= 1) & (k % NB == 0)
    if sp_first.any():
        sp_pos = np.flatnonzero(sp_first)
        sp_p = p_assign[ms[sp_pos]]
        so = np.argsort(sp_p, kind="stable")
        sp_sorted_p = sp_p[so]
        sp_starts = np.searchsorted(sp_sorted_p, np.arange(128))
        sp_within = np.arange(sp_pos.size) - sp_starts[sp_sorted_p]
        sp_bin = np.empty(sp_pos.size, np.int32)
        sp_bin[so] = n_prim[sp_sorted_p] + sp_within
        gid = np.cumsum(sp_first) - 1
        lvl_mask = level >= 1
        bins_sorted[lvl_mask] = sp_bin[gid[lvl_mask]]
        sp_mol = ms[sp_pos]
        sp_part = p_assign[sp_mol]
    else:
        sp_bin = np.empty(0, np.int32)
        sp_mol = np.empty(0, np.int32)
        sp_part = np.empty(0, np.int32)

    p_atom = p_assign[ms]
    o3 = np.lexsort((k, bins_sorted, p_atom))
    p3 = p_atom[o3]
    pstarts = np.searchsorted(p3, np.arange(128))
    pos = np.arange(n) - pstarts[p3]
    load = np.bincount(p3, minlength=128)
    if load.max() > T_COLS:
        raise RuntimeError(f"partition overload {load.max()} > {T_COLS}")
    nb_used = int(n_prim.max() + (np.bincount(sp_part, minlength=128).max()
                                  if sp_part.size else 0))
    if nb_used > BIN_HALF:
        raise RuntimeError(f"bins overload {nb_used} > {BIN_HALF}")

    batch = pos % NB
    col = batch * BW + pos // NB
    atom_ids = a_sort[o3]

    perm = np.zeros(SLOTS, np.int64)
    q = np.full((128, T_COLS), -1, np.int16)
    slot = col * 128 + p3
    perm[slot] = atom_ids
    q[p3, col] = bins_sorted[o3]

    bin_mol = np.concatenate([order, sp_mol])
    bin_p = np.concatenate([p_of_rank, sp_part])
    bin_id = np.concatenate([prim[order], sp_bin])
    return perm, q, bin_mol, bin_p, bin_id


# xt column for slot (p, col): the L3 emission order defines it.
def _slot_to_xt_col():
    p = np.arange(128)[:, None]
    c = np.arange(T_COLS)[None, :]
    jp = c // 32
    rr = c % 32
    j2 = rr // 16
    s = (rr % 16) // 4
    qq = rr % 4
    sb = qq // 2
    blk = qq % 2
    return (4096 * jp + 2048 * j2 + 1024 * sb + 512 * blk + 128 * s + p)


def _prep_weights(W1, b1, W2, b2, W3):
    w1 = np.ascontiguousarray(W1, np.float16)                       # [128, 64]
    w2s = np.zeros((128, 64), np.float32)                           # block-diag
    w2s[0:64, 0:32] = W2
    w2s[64:128, 32:64] = W2
    w3s = np.zeros((128, 4), np.float32)
    w3 = np.asarray(W3)[:, 0]
    for q in range(4):
        w3s[32 * q:32 * q + 32, q] = w3
    wpk = np.ascontiguousarray(
        np.hstack([w1.astype(np.float32), w2s, w3s]), np.float16)   # [128, 132]
    b1c = np.asarray(b1, np.float32).reshape(-1, 1)
    b2c = np.asarray(b2, np.float32).reshape(-1, 1)
    b1s = np.vstack([b1c, b1c]).astype(np.float32)                  # [128, 1]
    b2s = np.vstack([b2c] * 4).astype(np.float32)                   # [128, 1]
    return wpk, b1s, b2s


def _fit_poly(feats, W1, b1, W2, b2):
    """Weighted LSQ fit of deg-5 odd poly to tanh on the actual z2 sample."""
    x8 = feats[::29].astype(E4NP).astype(np.float32)
    z1 = x8 @ np.asarray(W1).astype(np.float16).astype(np.float32) + np.asarray(b1)
    h1 = np.tanh(z1).astype(np.float16).astype(np.float32)
    z2 = h1 @ np.asarray(W2).astype(np.float16).astype(np.float32) + np.asarray(b2)
    zs = z2.ravel()
    A = np.stack([zs, zs**3, zs**5], 1)
    c, *_ = np.linalg.lstsq(A, np.tanh(zs), rcond=None)
    return c.astype(np.float64)


# ================================================================ entry
def _prepare(
    feats_H, feats_O, mol_idx_H, mol_idx_O,
    W1_H, b1_H, W2_H, b2_H, W3_H,
    W1_O, b1_O, W2_O, b2_O, W3_O,
):
    feats = {"h": np.asarray(feats_H), "o": np.asarray(feats_O)}
    mols = {
        "h": np.asarray(mol_idx_H, np.int32),
        "o": np.asarray(mol_idx_O, np.int32),
    }
    raw = {
        "h": (W1_H, b1_H, W2_H, b2_H, W3_H),
        "o": (W1_O, b1_O, W2_O, b2_O, W3_O),
    }
    wts = {}
    c2 = {}
    for e in ("h", "o"):
        W1, b1, W2, b2, W3 = raw[e]
        wpk, b1s, b2s = _prep_weights(W1, b1, W2, b2, W3)
        c = _fit_poly(feats[e], W1, b1, W2, b2)
        c0s = np.full((128, 1), c[0], np.float32)
        c1s = np.full((128, 1), c[1], np.float32)
        bpk = np.ascontiguousarray(np.hstack([b1s, b2s, c0s, c1s]), np.float32)
        wts[e] = (wpk, bpk)
        c2[e] = float(c[2])

    xcol = _slot_to_xt_col()                       # [128, T_COLS]
    n16 = K16 * DMA_COLS

    in_maps = []
    merge = []
    for cidx in range(N_CORES):
        im = {}
        mg = []
        q_full = np.empty((128, 2 * T_COLS), np.int16)
        for ei, e in enumerate(("h", "o")):
            sl = slice(cidx * APC, (cidx + 1) * APC)
            perm, q, bm, bp, bid = _plan_element(mols[e][sl])
            # remap: perm indexed by (col*128 + p) -> xt-column order
            perm_xt = np.empty(SLOTS, np.int64)
            slot_idx = (np.arange(T_COLS)[None, :] * 128
                        + np.arange(128)[:, None])        # [128, T_COLS]
            perm_xt[xcol.ravel()] = perm[slot_idx.ravel()]
            g = feats[e][sl][perm_xt]                     # [SLOTS, 128] f32
            if n16:
                im[f"xt16_{e}"] = np.ascontiguousarray(
                    g[:n16].astype(np.float16).T)
            im[f"xt8_{e}"] = np.ascontiguousarray(g[n16:].astype(E4NP).T)
            del g
            off = ei * BIN_HALF
            q_full[:, ei * T_COLS:(ei + 1) * T_COLS] = q   # bins in [0, 640)
            mg.append((bm, bp, bid + off))
            im[f"wpk_{e}"], im[f"bpk_{e}"] = wts[e]
        im["q_idx"] = q_full
        in_maps.append(im)
        merge.append(mg)
    return in_maps, merge, c2


def kernel(
    feats_H, feats_O, mol_idx_H, mol_idx_O, n_molecules,
    W1_H, b1_H, W2_H, b2_H, W3_H, b3_H,
    W1_O, b1_O, W2_O, b2_O, W3_O, b3_O,
):
    from concourse import bass_utils

    in_maps, merge, c2 = _prepare(
        feats_H, feats_O, mol_idx_H, mol_idx_O,
        W1_H, b1_H, W2_H, b2_H, W3_H,
        W1_O, b1_O, W2_O, b2_O, W3_O,
    )
    key = ("nc", round(c2["h"], 9), round(c2["o"], 9), K16)
    if _CACHE.get("key") != key:
        _CACHE["nc"] = _build_nc(c2, K16)
        _CACHE["key"] = key
    nc = _CACHE["nc"]

    _CACHE["in_maps"] = in_maps
    res = bass_utils.run_bass_kernel_spmd(
        nc, in_maps, core_ids=list(range(N_CORES))
    )

    mols = {
        "h": np.asarray(mol_idx_H, np.int32),
        "o": np.asarray(mol_idx_O, np.int32),
    }
    out = np.zeros(N_MOL, np.float64)
    for c in range(N_CORES):
        acc = res.results[c]["out_acc"]
        for bm, bp, bid in merge[c]:
            out += np.bincount(
                bm, weights=acc[bp, bid].astype(np.float64), minlength=N_MOL
            )
    cnt_h = np.bincount(mols["h"], minlength=N_MOL)
    cnt_o = np.bincount(mols["o"], minlength=N_MOL)
    out += cnt_h * float(np.asarray(b3_H).reshape(-1)[0])
    out += cnt_o * float(np.asarray(b3_O).reshape(-1)[0])
    return out.astype(np.float32)
